# revision 1
# baseline (speedup 1.0000x reference)
"""GraphSAGE 2-layer (SAGEConv mean-aggregation) Bass kernel for 8 TRN2 NeuronCores.

Strategy (see spec sharding_hint):
  - Destination nodes sharded across 8 cores (12500/core). Within each core a
    greedy balancer assigns nodes to 98 windows x 128 slots so that each
    (window, src-block) cell has <= ~512 edges -> near-uniform SPMD schedule.
  - Edges partitioned by destination core, sorted by (window, src-block) and
    by source row within each cell (HBM locality for the gathers).
  - Aggregation: dma_gather pulls rows (bf16) from HBM in 4 source blocks
    (int16 index limit); selection tiles (pure 0/1 one-hot, [128 edges x 128
    slots]) are streamed pre-built from HBM in fp8 (half the bytes of bf16,
    exactly representable); TensorE accumulates raw neighbor sums^T per
    window in PSUM. The mean division (invdeg per destination slot) is
    applied by one DVE tensor_tensor(mult) per window against an SBUF-
    resident [128, SLOTS] broadcast tile of per-slot inverse degrees.
  - Transform per window: two 128x128 matmuls (+ bias) produce hT; layer-1 h
    is transposed to row-major SLOT order and written with plain DMA (no
    scatter). One AllGather of the slot-ordered shard forms hfull; layer-2
    gathers reference (core, slot) coordinates directly - 2*NSH == BS so
    layer-2 blocks coincide with layer-1 source blocks and the cell
    structure is shared.
  - Final layer-2 output is written in slot order (bf16) and inverse-
    permuted on host.
"""

import sys

sys.path.insert(0, "/opt/trn_rl_repo")

from contextlib import ExitStack
from dataclasses import dataclass

import ml_dtypes
import numpy as np

import concourse.bacc as bacc
import concourse.bass as bass
import concourse.mybir as mybir
import concourse.tile as tile
from concourse.bass_utils import run_bass_kernel_spmd

BF = mybir.dt.bfloat16
F32 = mybir.dt.float32
I16 = mybir.dt.int16
bfnp = ml_dtypes.bfloat16
f8np = ml_dtypes.float8_e4m3
FP8 = mybir.dt.float8e4


@dataclass
class Cfg:
    N: int = 100000      # total nodes
    D: int = 128         # feature dim
    C: int = 8           # cores
    NB: int = 4          # source blocks (int16 gather index limit)
    WN: int = 98         # windows per core (128 dst nodes each)
    CALL: int = 1024     # gather indices per dma_gather call
    SCW: int = 4         # windows per h-store dma
    OCW: int = 4         # windows per final output dma
    G: int = 1           # AllGather chunks (window groups)
    DVE_EVERY: int = 0   # 1 of every DVE_EVERY sel-tiles built on DVE (0: none)
    ACT_EVERY: int = 0   # 1 of every ACT_EVERY sel-tiles built on ScalarE (0: none)

    @property
    def NSH(self):
        return self.N // self.C

    @property
    def BS(self):
        return self.N // self.NB

    @property
    def SLOTS(self):
        return self.WN * 128

    @property
    def CALLCH(self):
        return self.CALL // 128

    @property
    def WCH(self):
        # window group boundaries for AllGather chunks
        per = -(-self.WN // self.G)
        return [min(self.WN, per * g) for g in range(self.G + 1)]


CFG = Cfg()


# ---------------------------------------------------------------- host prep


def _balance_core(dnb, WN, cap=128, ctarget=512):
    """Assign nodes (rows of dnb, per-block in-degree vectors) to WN bins of
    <=cap nodes, aiming for per-(bin, block) sums <= target. Overflow (when a
    block's total exceeds WN*ctarget) is concentrated in the LAST windows.
    Returns (bin id per node, binsum)."""
    nn, NB = dnb.shape
    T = dnb.sum(0)
    # per-block overflow chunks, assigned to tail windows
    target = np.full((WN, NB), ctarget, np.int64)
    for b in range(NB):
        q = max(0, -(-int(T[b] - WN * ctarget) // 128))
        for i in range(min(q, WN)):
            target[WN - 1 - i, b] += 128
    tot = dnb.sum(1)
    order = np.argsort(-tot, kind="stable")
    binsum = np.zeros((WN, NB), np.int64)
    binslots = np.zeros(WN, np.int64)
    assign = np.full(nn, -1, np.int64)
    tgt = target.astype(np.float64)
    for n in order:
        dv = dnb[n]
        fill = ((binsum + dv) / tgt).max(axis=1)
        fill += 1e-5 * binslots
        fill[binslots >= cap] = 1e30
        j = int(np.argmin(fill))
        assign[n] = j
        binsum[j] += dv
        binslots[j] += 1

    # repair: evict small-degree nodes from violated cells into bins with
    # slack (move if a slot is free, else swap with a light partner)
    for _ in range(30):
        viol = np.argwhere(binsum > target)
        if len(viol) == 0:
            break
        moved = 0
        for j, b in viol:
            guard = 0
            while binsum[j, b] > target[j, b] and guard < 64:
                guard += 1
                members = np.where(assign == j)[0]
                mb = dnb[members, b]
                cand_n = members[mb > 0]
                if len(cand_n) == 0:
                    break
                # smallest positive contribution first
                cand_n = cand_n[np.argsort(dnb[cand_n, b], kind="stable")]
                done = False
                for n in cand_n[:8]:
                    dv = dnb[n]
                    ok = ((binsum + dv) <= target).all(axis=1) & (binslots < cap)
                    ok[j] = False
                    cand = np.where(ok)[0]
                    if len(cand):
                        j2 = int(cand[np.argmin(((binsum[cand] + dv) / target[cand]).max(1))])
                        assign[n] = j2
                        binsum[j] -= dv
                        binsum[j2] += dv
                        binslots[j] -= 1
                        binslots[j2] += 1
                        moved += 1
                        done = True
                        break
                    # swap with the lightest partner in low-fill bins
                    for j2 in np.argsort(binsum[:, b])[:24]:
                        if j2 == j:
                            continue
                        mem2 = np.where(assign == j2)[0]
                        if len(mem2) == 0:
                            continue
                        m = mem2[np.argmin(dnb[mem2, b])]
                        dm = dnb[m]
                        if dm[b] >= dv[b]:
                            continue
                        nj = binsum[j] - dv + dm
                        nj2 = binsum[j2] - dm + dv
                        if (nj <= target[j]).all() and (nj2 <= target[j2]).all():
                            assign[n], assign[m] = j2, j
                            binsum[j] = nj
                            binsum[j2] = nj2
                            moved += 1
                            done = True
                            break
                    if done:
                        break
                if not done:
                    break
        if moved == 0:
            break
    return assign, binsum


def _layer_sched(counts, cfg):
    """counts: [C, WN, NBL] per-core cell sizes -> shared schedule dict."""
    K = np.ceil(counts / 128).astype(np.int64).max(axis=0)  # [WN, NBL]
    TCH = int(K.sum())
    Sb = (K.sum(axis=0) * 128).astype(np.int64)             # [NBL] idx slots
    ncalls = np.ceil(Sb / cfg.CALL).astype(np.int64)
    lastvalid = Sb - (ncalls - 1) * cfg.CALL
    # DVE / streamed-sval split, round-robin by chunk column
    if cfg.DVE_EVERY > 0:
        is_dve = (np.arange(TCH) % cfg.DVE_EVERY) == 0
    else:
        is_dve = np.zeros(TCH, bool)
    if cfg.ACT_EVERY > 0:
        is_act = (~is_dve) & ((np.arange(TCH) % cfg.ACT_EVERY) == 1)
    else:
        is_act = np.zeros(TCH, bool)
    is_dve = is_dve | is_act                 # "on-chip" tiles (drel-driven)
    dcol_of = np.cumsum(is_dve) - 1          # on-chip drel column index
    svi_of = np.cumsum(~is_dve) - 1          # streamed sval tile index
    NDV = int(is_dve.sum())
    NSV = TCH - NDV
    NBL = K.shape[1]
    GCOLS_B = [int(ncalls[b]) * (cfg.CALL // 16) for b in range(NBL)]
    GOFF = np.concatenate([[0], np.cumsum(GCOLS_B)]).astype(int)
    return dict(K=K, TCH=TCH, Sb=Sb, ncalls=ncalls, lastvalid=lastvalid,
                is_dve=is_dve, is_act=is_act, dcol_of=dcol_of, svi_of=svi_of,
                NDV=NDV, NSV=max(NSV, 1), GOFF=GOFF)


def _core_layer_maps(sch, cellcnt, rel_src, ep, ev, cfg):
    """Build gather idx stream + per-chunk metadata for one (core, layer).
    rel_src: int64 block-relative row index per edge (cell-sorted order).
    Returns dict with gidx [128, GCOLS], drel/vval [128, NDV] f32,
    sval [128, NSV*128] bf16."""
    WN, CALL = cfg.WN, cfg.CALL
    K, TCH, Sb, ncalls = sch["K"], sch["TCH"], sch["Sb"], sch["ncalls"]
    is_dve, dcol_of, svi_of = sch["is_dve"], sch["dcol_of"], sch["svi_of"]
    NDV, NSV = sch["NDV"], sch["NSV"]
    NBL = K.shape[1]

    gstreams = [np.zeros(int(ncalls[b]) * CALL, np.int16) for b in range(NBL)]
    for b in range(NBL):
        if Sb[b] < ncalls[b] * CALL:
            gstreams[b][Sb[b]:] = -1  # tail of last call: skipped by DMA
    drel = np.full((max(NDV, 1), 128), -255.0, np.float32)
    vval = np.zeros((max(NDV, 1), 128), np.float32)
    sval = np.zeros((NSV, 128, 128), f8np)

    eoff = 0
    gcol = 0
    posb = np.zeros(NBL, np.int64)
    for w in range(WN):
        for b in range(NBL):
            L = int(cellcnt[w, b])
            kwb = int(K[w, b])
            if kwb == 0:
                assert L == 0
                continue
            sl = slice(eoff, eoff + L)
            st = int(posb[b])
            gstreams[b][st:st + L] = rel_src[sl].astype(np.int16)
            epc = ep[sl]
            evc = ev[sl]
            for kk in range(kwb):
                r0, r1 = kk * 128, min((kk + 1) * 128, L)
                gc = gcol + kk
                if is_dve[gc]:
                    dc = int(dcol_of[gc])
                    if r1 > r0:
                        drel[dc, :r1 - r0] = -epc[r0:r1]
                        vval[dc, :r1 - r0] = evc[r0:r1].astype(np.float32)
                else:
                    si = int(svi_of[gc])
                    if r1 > r0:
                        sval[si, np.arange(r1 - r0), epc[r0:r1]] = 1.0
            posb[b] += kwb * 128
            gcol += kwb
            eoff += L
    assert gcol == TCH

    gparts = []
    for b in range(NBL):
        arr = gstreams[b].reshape(-1, 16).T  # [16, Sb_pad/16]
        gparts.append(arr)
    gidx16 = np.concatenate(gparts, axis=1)          # [16, GCOLS]
    gidx = np.tile(gidx16, (8, 1)).astype(np.int16)  # [128, GCOLS]
    return dict(
        gidx=gidx,
        drel=np.ascontiguousarray(drel.T), vval=np.ascontiguousarray(vval.T),
        sval=np.ascontiguousarray(sval.transpose(1, 0, 2).reshape(128, -1)),
    ), eoff


def prep(x, edge_index, cfg=CFG):
    """Host-side sharding/schedule. Returns (schedule, per-core input maps)."""
    C, NB, WN, NSH, BS = cfg.C, cfg.NB, cfg.WN, cfg.NSH, cfg.BS
    SLOTS, G = cfg.SLOTS, cfg.G
    WCH = cfg.WCH
    src = np.asarray(edge_index[0]).astype(np.int64)
    dst = np.asarray(edge_index[1]).astype(np.int64)

    deg = np.bincount(dst, minlength=cfg.N).astype(np.float64)
    invdeg = (1.0 / np.maximum(deg, 1.0)).astype(np.float32)
    vedge_all = invdeg[dst].astype(bfnp)

    ecore = dst // NSH
    eblock1 = src // BS

    # --- per-core balance: node-local id -> (window, pos)
    win_of = np.zeros(cfg.N, np.int64)   # window within core
    pos_of = np.zeros(cfg.N, np.int64)   # slot within window
    counts1 = np.zeros((C, WN, NB), np.int64)
    for c in range(C):
        lo = c * NSH
        dnb = np.zeros((NSH, NB), np.int64)
        emask = ecore == c
        np.add.at(dnb, (dst[emask] - lo, eblock1[emask]), 1)
        assign, binsum = _balance_core(dnb, WN)
        # order bins by descending per-block chunk tuple so heavy cells align
        # at the same window index across cores
        kt = np.ceil(binsum / 128).astype(np.int64)
        key = [tuple(-kt[j]) + tuple(-binsum[j]) for j in range(WN)]
        order = sorted(range(WN), key=lambda j: key[j])
        rank = np.empty(WN, np.int64)
        rank[order] = np.arange(WN)
        w = rank[assign]
        win_of[lo:lo + NSH] = w
        # position within window: stable by node id
        order2 = np.lexsort((np.arange(NSH), w))
        pos = np.zeros(NSH, np.int64)
        pcount = np.zeros(WN, np.int64)
        for m in order2:
            pos[m] = pcount[w[m]]
            pcount[w[m]] += 1
        pos_of[lo:lo + NSH] = pos
        cnt = np.zeros((WN, NB), np.int64)
        np.add.at(cnt, (w[dst[emask] - lo], eblock1[emask]), 1)
        counts1[c] = cnt

    # layer-2 geometry: hfull is chunk-major: for AllGather chunk g
    # (windows [WCH[g], WCH[g+1])), node row = cbase[g] + core*crows[g] +
    # (win - WCH[g])*128 + pos; gather block g covers rows
    # [cbase[g], cbase[g+1]).
    if G == 1:
        # core-major hfull: blocks of 2 cores (2*NSH == BS) share layer-1's
        # cell structure exactly
        g2_of = (np.arange(cfg.N) // NSH) * SLOTS + win_of * 128 + pos_of
        eblock2 = eblock1
        cbase = np.arange(NB + 1, dtype=np.int64) * 2 * SLOTS
        counts2 = counts1
    else:
        crows = np.array([(WCH[g + 1] - WCH[g]) * 128 for g in range(G)], np.int64)
        cbase = np.concatenate([[0], np.cumsum(crows * C)]).astype(np.int64)
        for g in range(G):
            assert crows[g] * C <= 32767, (g, crows[g] * C)
        gchunk_of_w = np.zeros(WN, np.int64)
        for g in range(G):
            gchunk_of_w[WCH[g]:WCH[g + 1]] = g
        wch_arr = np.asarray(WCH)
        vg = gchunk_of_w[win_of]
        g2_of = (cbase[vg] + (np.arange(cfg.N) // NSH) * crows[vg]
                 + (win_of - wch_arr[vg]) * 128 + pos_of)
        eblock2 = vg[src]
        counts2 = np.zeros((C, WN, G), np.int64)
        for c in range(C):
            emask = ecore == c
            np.add.at(counts2[c], (win_of[dst[emask]], eblock2[emask]), 1)

    sch1 = _layer_sched(counts1, cfg)
    sch2 = _layer_sched(counts2, cfg)

    ewin = win_of[dst]
    epos = pos_of[dst]

    in_maps = []
    for c in range(C):
        lo = c * NSH
        emask = ecore == c
        es = src[emask]
        ew, eb1, eb2 = ewin[emask], eblock1[emask], eblock2[emask]
        ep = epos[emask]
        ev = vedge_all[emask]
        eg2 = g2_of[es]

        # layer 1: cells (w, src//BS), sorted by src within cell
        o1 = np.lexsort((es, eb1, ew))
        m1, ne1 = _core_layer_maps(
            sch1, counts1[c], (es - eb1 * BS)[o1], ep[o1], ev[o1], cfg)
        assert ne1 == es.shape[0]
        # layer 2: cells (w, chunk(src)), sorted by hfull row within cell
        rel2 = eg2 - cbase[eb2]
        o2 = np.lexsort((rel2, eb2, ew))
        m2, ne2 = _core_layer_maps(
            sch2, counts2[c], rel2[o2], ep[o2], ev[o2], cfg)
        assert ne2 == es.shape[0]

        # slot s = win*128+pos of node-local rows; dummy slots unused
        msk = np.arange(cfg.N)[lo:lo + NSH]
        sl_idx = win_of[msk] * 128 + pos_of[msk]
        sl_nodes = np.full(cfg.SLOTS, -1, np.int64)
        sl_nodes[sl_idx] = np.arange(NSH)

        # xT in slot order
        xT = np.zeros((cfg.D, cfg.SLOTS), bfnp)
        xT[:, sl_idx] = np.asarray(x[lo:lo + NSH]).astype(bfnp).T
        invsl = np.ones((cfg.SLOTS,), np.float32)
        invsl[sl_idx] = invdeg[lo:lo + NSH]
        bcast = np.ascontiguousarray(
            np.broadcast_to(invsl.astype(bfnp), (128, cfg.SLOTS)))

        in_maps.append(dict(
            gidx1=m1["gidx"], dstrel1=m1["drel"], vval1=m1["vval"], sval1=m1["sval"],
            gidx2=m2["gidx"], dstrel2=m2["drel"], vval2=m2["vval"], sval2=m2["sval"],
            xT=np.ascontiguousarray(xT), bcast=bcast,
            slot_nodes=sl_nodes,                   # host-only
        ))

    sched = dict(sch1=sch1, sch2=sch2, cbase=cbase,
                 TCH=sch1["TCH"] + sch2["TCH"])
    return sched, in_maps


# ---------------------------------------------------------------- program


def build(cfg, sched):
    sch = {1: sched["sch1"], 2: sched["sch2"]}
    cbase = sched["cbase"]
    C, D, NB, WN, BS = cfg.C, cfg.D, cfg.NB, cfg.WN, cfg.BS
    CALL, CALLCH, SLOTS, G = cfg.CALL, cfg.CALLCH, cfg.SLOTS, cfg.G
    WCH = cfg.WCH

    nc = bacc.Bacc(None, num_devices=C, num_swdge_queues=4,
                   dynamic_dma_scratch_size=24576)
    x_d = nc.dram_tensor("xbf", [cfg.N, D], BF, kind="ExternalInput")
    xT_d = nc.dram_tensor("xT", [D, SLOTS], BF, kind="ExternalInput")
    gidx_d, drel_d, vval_d, sval_d = {}, {}, {}, {}
    for L in (1, 2):
        s = sch[L]
        gidx_d[L] = nc.dram_tensor(f"gidx{L}", [128, int(s["GOFF"][-1])], I16,
                                   kind="ExternalInput")
        drel_d[L] = nc.dram_tensor(f"dstrel{L}", [128, max(s["NDV"], 1)], F32,
                                   kind="ExternalInput")
        vval_d[L] = nc.dram_tensor(f"vval{L}", [128, max(s["NDV"], 1)], F32,
                                   kind="ExternalInput")
        sval_d[L] = nc.dram_tensor(f"sval{L}", [128, s["NSV"] * 128], FP8,
                                   kind="ExternalInput")
    w_d = {}
    for nm in ("wlt1", "wrt1", "wlt2", "wrt2"):
        w_d[nm] = nc.dram_tensor(nm, [D, D], BF, kind="ExternalInput")
    b1_d = nc.dram_tensor("b1c", [D, 1], F32, kind="ExternalInput")
    b2_d = nc.dram_tensor("b2r", [1, D], F32, kind="ExternalInput")
    bc_d = nc.dram_tensor("bcast", [128, SLOTS], BF, kind="ExternalInput")
    out_d = nc.dram_tensor("out", [SLOTS, D], BF, kind="ExternalOutput")

    ident_d = nc.inline_tensor(np.eye(128, dtype=bfnp), "identc")
    iota_d = nc.inline_tensor(
        np.broadcast_to(np.arange(128, dtype=bfnp), (128, 128)).copy(), "iotac")
    ones_d = nc.inline_tensor(np.ones((1, 128), np.float32), "onesc")

    hsl_d = nc.dram_tensor("hslots", [SLOTS, D], BF)  # Internal, slot order
    hfull_d = nc.dram_tensor("hfull", [C * SLOTS, D], BF, addr_space="Shared")

    with tile.TileContext(nc) as tc, ExitStack() as ctx:
        const = ctx.enter_context(tc.tile_pool(name="const", bufs=1))
        meta = ctx.enter_context(tc.tile_pool(name="meta", bufs=1))
        gpool = ctx.enter_context(tc.tile_pool(name="gather", bufs=6))
        spool = ctx.enter_context(tc.tile_pool(name="sv", bufs=6))
        ohp = ctx.enter_context(tc.tile_pool(name="oh", bufs=1))
        dhp = ctx.enter_context(tc.tile_pool(name="dh", bufs=1))
        mwp = ctx.enter_context(tc.tile_pool(name="mw", bufs=6))
        htp = ctx.enter_context(tc.tile_pool(name="ht", bufs=1))
        xtp = ctx.enter_context(tc.tile_pool(name="xt", bufs=2))
        stgp = ctx.enter_context(tc.tile_pool(name="stg", bufs=2))
        ostgp = ctx.enter_context(tc.tile_pool(name="ostg", bufs=2))
        psA = ctx.enter_context(tc.tile_pool(name="psA", bufs=4, space="PSUM"))
        psB = ctx.enter_context(tc.tile_pool(name="psB", bufs=2, space="PSUM"))
        psT = ctx.enter_context(tc.tile_pool(name="psT", bufs=2, space="PSUM"))
        bcp = ctx.enter_context(tc.tile_pool(name="bc", bufs=1))

        def load(pool, dram, shape, dtype):
            t = pool.tile(shape, dtype, tag=dram.name)
            nc.sync.dma_start(t[:], dram[:])
            return t

        ident_s = load(const, ident_d, [128, 128], BF)
        iota_s = load(const, iota_d, [128, 128], BF)
        ones_s = load(const, ones_d, [1, 128], F32)
        w_s = {nm: load(const, w_d[nm], [D, D], BF) for nm in w_d}
        b1_s = load(const, b1_d, [D, 1], F32)
        b2_s = load(const, b2_d, [1, D], F32)

        drel_s = {L: load(meta, drel_d[L], [128, max(sch[L]["NDV"], 1)], F32)
                  for L in (1, 2)}
        vval_s = {L: load(meta, vval_d[L], [128, max(sch[L]["NDV"], 1)], F32)
                  for L in (1, 2)}
        gidx_s = {}
        for L in (1, 2):
            gidx_s[L] = meta.tile([128, int(sch[L]["GOFF"][-1])], I16,
                                  tag=f"gidx{L}", name=f"gidx{L}")

        def load_gidx(L):
            GOFF = sch[L]["GOFF"]
            for _b in range(sch[L]["K"].shape[1]):
                nc.sync.dma_start(gidx_s[L][:, int(GOFF[_b]):int(GOFF[_b + 1])],
                                  gidx_d[L][:, int(GOFF[_b]):int(GOFF[_b + 1])])

        load_gidx(1)
        # prefetch the first sval groups of layer 1 ahead of the bulk loads
        prefetched = {}
        for _si in range(2):
            _nch = min(8, sch[1]["NSV"] - _si * 8)
            _stt = spool.tile([128, 8, 128], FP8, tag="sv", name=f"sv1_{_si}")
            nc.sync.dma_start(_stt[:, :_nch, :],
                              sval_d[1][:, _si * 1024:_si * 1024 + _nch * 128])
            prefetched[_si] = _stt
        bc_s = load(bcp, bc_d, [128, SLOTS], BF)
        hT_s = htp.tile([D, SLOTS], BF, tag="hT")

        def run_layer(L, w_lo, w_hi):
            s = sch[L]
            K, ncalls, lastvalid = s["K"], s["ncalls"], s["lastvalid"]
            is_dve, dcol_of, svi_of, NSV = (s["is_dve"], s["dcol_of"],
                                            s["svi_of"], s["NSV"])
            GOFF = s["GOFF"]
            NBL = K.shape[1]
            src_d = x_d if L == 1 else hfull_d
            drl, vvl, gix = drel_s[L], vval_s[L], gidx_s[L]
            st_ = run_layer.state.setdefault(
                L, dict(posb=[0] * NBL, gt=[None] * NBL, gcol=0, stile=None,
                        stg=None, ostg=None, bct=None, xtt=None))
            posb, gt = st_["posb"], st_["gt"]
            SCW, OCW = cfg.SCW, cfg.OCW
            for w in range(w_lo, w_hi):
                nchunks_w = int(K[w].sum())
                psum_a = psA.tile([128, 128], F32, tag="agg", name=f"agg{L}_{w}")
                ci = 0
                for b in range(NBL):
                    for k in range(int(K[w, b])):
                        pos = posb[b]
                        call_i, col = divmod(pos, CALLCH)
                        if col == 0:
                            gt[b] = gpool.tile([128, CALLCH, 128], BF, tag=f"g{b}",
                                               name=f"g{b}_{L}_{call_i}")
                            nvalid = CALL if call_i < int(ncalls[b]) - 1 else int(lastvalid[b])
                            ioff = GOFF[b] + call_i * (CALL // 16)
                            if L == 1:
                                in_ap = src_d[b * BS:(b + 1) * BS, :]
                            else:
                                in_ap = src_d[int(cbase[b]):int(cbase[b + 1]), :]
                            nc.gpsimd.dma_gather(
                                out_ap=gt[b][:],
                                in_ap=in_ap,
                                idxs_ap=gix[:, ioff:ioff + CALL // 16],
                                num_idxs=CALL,
                                num_idxs_reg=nvalid,
                                elem_size=D,
                            )
                        gcol = st_["gcol"]
                        if is_dve[gcol]:
                            dc = int(dcol_of[gcol])
                            dd = dhp.tile([128, 128], BF, tag="dh",
                                          name=f"dh{L}_{gcol}")
                            nc.scalar.activation(
                                dd[:], iota_s[:],
                                mybir.ActivationFunctionType.Abs,
                                bias=drl[:, dc:dc + 1], scale=1.0)
                            S = ohp.tile([128, 128], BF, tag="oh",
                                         name=f"oh{L}_{gcol}")
                            nc.scalar.activation(
                                S[:], dd[:],
                                mybir.ActivationFunctionType.Relu,
                                bias=1.0, scale=-1.0)
                            rhs_ap = S[:]
                        else:
                            si, sc = divmod(int(svi_of[gcol]), 8)
                            if sc == 0:
                                if L == 1 and si in prefetched:
                                    st_["stile"] = prefetched.pop(si)
                                elif (L, si) in prefetched:
                                    st_["stile"] = prefetched.pop((L, si))
                                else:
                                    nch = min(8, NSV - si * 8)
                                    stt = spool.tile([128, 8, 128], FP8, tag="sv",
                                                     name=f"sv{L}_{si}")
                                    nc.sync.dma_start(
                                        stt[:, :nch, :],
                                        sval_d[L][:, si * 1024:si * 1024 + nch * 128])
                                    st_["stile"] = stt
                            rhs_ap = st_["stile"][:, sc, :]
                        nc.tensor.matmul(
                            out=psum_a[:], lhsT=gt[b][:, col, :], rhs=rhs_ap,
                            start=(ci == 0), stop=(ci == nchunks_w - 1),
                        )
                        st_["gcol"] += 1
                        posb[b] += 1
                        ci += 1
                m_s = mwp.tile([128, 128], BF, tag="mw", name=f"mw{L}_{w}")
                wsl = slice(w * 128, (w + 1) * 128)
                if nchunks_w:
                    nc.vector.tensor_tensor(out=m_s[:], in0=psum_a[:],
                                            in1=bc_s[:, wsl],
                                            op=mybir.AluOpType.mult)
                else:
                    nc.vector.memset(m_s[:], 0.0)
                if L == 1:
                    psum_h = psB.tile([128, 128], F32, tag="h", name=f"h{L}_{w}")
                    nc.tensor.matmul(out=psum_h[:], lhsT=w_s["wlt1"][:], rhs=m_s[:],
                                     start=True, stop=False)
                    xgi, xgo = divmod(w, 8)
                    if xgo == 0 or st_["xtt"] is None:
                        nxw = min(8, WN - xgi * 8)
                        xtt = xtp.tile([128, 8, 128], BF, tag="xT",
                                       name=f"xT{xgi}")
                        nc.sync.dma_start(
                            xtt[:, :nxw, :],
                            xT_d[:, xgi * 1024:xgi * 1024 + nxw * 128])
                        st_["xtt"] = xtt
                    nc.tensor.matmul(out=psum_h[:], lhsT=w_s["wrt1"][:],
                                     rhs=st_["xtt"][:, xgo, :], start=False,
                                     stop=True)
                    nc.scalar.activation(hT_s[:, wsl], psum_h[:],
                                         mybir.ActivationFunctionType.Identity,
                                         bias=b1_s[:, 0:1], scale=1.0)
                    psum_t = psT.tile([128, 128], BF, tag="tr", name=f"tr{w}")
                    nc.tensor.transpose(psum_t[:], hT_s[:, wsl], ident_s[:])
                    wi = (w - w_lo) % SCW
                    if wi == 0:
                        st_["stg"] = stgp.tile([128, SCW, 128], BF, tag="stg",
                                               name=f"stg{w}")
                    nc.scalar.copy(st_["stg"][:, wi, :], psum_t[:])
                    if wi == SCW - 1 or w == w_hi - 1:
                        used = wi + 1
                        w0 = w - wi
                        hap = hsl_d[:].rearrange("(w p) f -> p w f", p=128)
                        nc.sync.dma_start(hap[:, w0:w0 + used, :],
                                          st_["stg"][:, :used, :])
                else:
                    psum_h = psB.tile([128, 128], F32, tag="h", name=f"h{L}_{w}")
                    nc.tensor.matmul(out=psum_h[:], lhsT=m_s[:], rhs=w_s["wlt2"][:],
                                     start=True, stop=False)
                    nc.tensor.matmul(out=psum_h[:], lhsT=hT_s[:, wsl],
                                     rhs=w_s["wrt2"][:], start=False, stop=False)
                    nc.tensor.matmul(out=psum_h[:], lhsT=ones_s[0:1, :],
                                     rhs=b2_s[0:1, :], start=False, stop=True)
                    wi = w % OCW
                    if wi == 0:
                        st_["ostg"] = ostgp.tile([128, OCW, 128], BF, tag="ostg",
                                                 name=f"ostg{w}")
                    nc.scalar.copy(st_["ostg"][:, wi, :], psum_h[:])
                    if wi == OCW - 1 or w == WN - 1:
                        used = wi + 1
                        w0 = w - wi
                        oap = out_d[:].rearrange("(w p) f -> p w f", p=128)
                        nc.sync.dma_start(oap[:, w0:w0 + used, :],
                                          st_["ostg"][:, :used, :])

        run_layer.state = {}
        # layer 1 in window chunks; AllGather each chunk as soon as its
        # windows are stored (overlaps the collective with remaining compute)
        for g in range(G):
            run_layer(1, WCH[g], WCH[g + 1])
            if g == G - 1:
                load_gidx(2)
                for _si in range(4):
                    _nch = min(8, sch[2]["NSV"] - _si * 8)
                    _stt = spool.tile([128, 8, 128], FP8, tag="sv",
                                      name=f"sv2_{_si}")
                    nc.sync.dma_start(
                        _stt[:, :_nch, :],
                        sval_d[2][:, _si * 1024:_si * 1024 + _nch * 128])
                    prefetched[(2, _si)] = _stt
            r0, r1 = WCH[g] * 128, WCH[g + 1] * 128
            out_lo = 0 if G == 1 else int(cbase[g])
            out_hi = C * SLOTS if G == 1 else int(cbase[g + 1])
            nc.gpsimd.collective_compute(
                "AllGather", mybir.AluOpType.bypass,
                replica_groups=[list(range(C))],
                ins=[hsl_d[r0:r1, :]],
                outs=[hfull_d[out_lo:out_hi, :]],
            )
        run_layer(2, 0, WN)

    # spread SWDGE gather descriptor generation across the 4 SWDGE queues
    # (parallel Q7 pairs). Tile assigned DMASW lanes round-robin in scheduled
    # order; keep sem-lane <-> queue binding consistent by deriving the queue
    # from the lane (lane % 4).
    from concourse.tile_sem_assignment import PROC_NAME_TO_IDX
    dmasw0 = PROC_NAME_TO_IDX["DMASW0"]
    for inst in nc.inst_map.values():
        if isinstance(inst, (mybir.InstDMAGatherAnt, mybir.InstDMAScatterAddAnt)):
            proc = getattr(inst, "bass_scheduled_proc", None)
            if proc is not None and dmasw0 <= proc < dmasw0 + 8:
                inst.queue_num = (proc - dmasw0) % 4

    nc.compile()
    return nc


# ---------------------------------------------------------------- kernel


def kernel(**inputs):
    cfg = CFG
    x = np.asarray(inputs["x"], np.float32)
    ei = np.asarray(inputs["edge_index"])
    sched, in_maps = prep(x, ei, cfg)
    nc = build(cfg, sched)

    x_bf = x.astype(bfnp)
    shared = dict(
        xbf=x_bf,
        wlt1=np.ascontiguousarray(np.asarray(inputs["Wl1"], np.float32).T.astype(bfnp)),
        wrt1=np.ascontiguousarray(np.asarray(inputs["Wr1"], np.float32).T.astype(bfnp)),
        wlt2=np.ascontiguousarray(np.asarray(inputs["Wl2"], np.float32).T.astype(bfnp)),
        wrt2=np.ascontiguousarray(np.asarray(inputs["Wr2"], np.float32).T.astype(bfnp)),
        b1c=np.asarray(inputs["b1"], np.float32).reshape(cfg.D, 1).copy(),
        b2r=np.asarray(inputs["b2"], np.float32).reshape(1, cfg.D).copy(),
    )
    slot_nodes = [m.pop("slot_nodes") for m in in_maps]
    run_maps = [dict(shared, **{k: v for k, v in m.items()}) for m in in_maps]

    res = None
    last_err = None
    for attempt in range(3):
        try:
            res = run_bass_kernel_spmd(nc, run_maps, core_ids=list(range(cfg.C)))
            break
        except Exception as e:  # transient device wedge: retry
            last_err = e
            import time
            time.sleep(10)
    if res is None:
        raise last_err
    out = np.empty((cfg.N, cfg.D), np.float32)
    for c in range(cfg.C):
        oc = res.results[c]["out"]
        sn = slot_nodes[c]
        real = sn >= 0
        out[c * cfg.NSH + sn[real]] = oc[real]
    return out


if __name__ == "__main__":
    d = np.load("/tmp/inputs.npz")
    ins = {k: d[k] for k in ("x", "edge_index", "Wl1", "Wr1", "b1", "Wl2", "Wr2", "b2")}
    got = kernel(**ins)
    exp = d["expected"]
    err = np.abs(got - exp).max() / np.abs(exp).max()
    print("Relative error:", err)



# revision 20
# speedup vs baseline: 1.0241x; 1.0241x over previous
"""GraphSAGE 2-layer Bass kernel for 8 TRN2 NeuronCores — v2.

Algebraic refactor: with A = mean-aggregation (D^-1 Adj),
  h   = A(x)@Wl1.T + b1 + x@Wr1.T
  out = A(h)@Wl2.T + b2 + h@Wr2.T
      = mm@WaT + m@WbT + x@WcT + 1_{deg>0} (b1@Wl2.T) + (b2 + b1@Wr2.T)
where m = A(x), mm = A(m), Wa = Wl2@Wl1, Wb = Wl2@Wr1 + Wr2@Wl1,
Wc = Wr2@Wr1 (host-folded, f64).

Device structure per core (12544 slots = 49 groups x 256, destination
nodes degree-sorted into slots):
  L1: host pre-gathers x[src] per edge into a sequential bf16 stream in
      "identity one-hot" order (group, round, slot) -> aggregation is a
      plain accumulate-transpose matmul against a constant identity rhs.
      No SWDGE descriptors, no selection tiles. m = psum * invdeg.
  AllGather of m in G=4 slot-range chunks (chunk-major mfull layout) so
      L2 gathers for chunk b start while L1 computes chunk b+1.
  L2: dma_gather of m rows per edge (SWDGE, 4 queues), cells are
      (dest-group, source-chunk); one-hot tiles built on-chip by batched
      DVE is_equal (iota vs per-edge slot position, stride-0 broadcast
      APs). Partials accumulated across source chunks in an SBUF f32
      accumulator, then the 3-matmul transform writes the output.
"""

import sys

sys.path.insert(0, "/opt/trn_rl_repo")

from contextlib import ExitStack
from dataclasses import dataclass

import ml_dtypes
import numpy as np

import concourse.bacc as bacc
import concourse.bass as bass
import concourse.mybir as mybir
import concourse.tile as tile
from concourse.bass_utils import run_bass_kernel_spmd

BF = mybir.dt.bfloat16
F32 = mybir.dt.float32
I16 = mybir.dt.int16
bfnp = ml_dtypes.bfloat16


@dataclass
class Cfg:
    N: int = 100000      # total nodes
    D: int = 128         # feature dim
    C: int = 8           # cores
    GRP: int = 256       # slots per destination group (psum width)
    NG: int = 49         # groups per core
    G: int = 4           # AllGather chunks == L2 gather blocks
    CALL: int = 1024     # gather indices per dma_gather call
    XEG: int = 32        # L1 stream chunks per DMA load
    B2: int = 4          # one-hot chunks built per op batch
    SCAL_EVERY: int = 4  # 1 of every N one-hot batches goes to ScalarE
    SCW: int = 4         # 128-windows per m-store dma
    OCW: int = 4         # 128-windows per output dma

    @property
    def NSH(self):
        return self.N // self.C

    @property
    def SLOTS(self):
        return self.NG * self.GRP

    @property
    def CALLCH(self):
        return self.CALL // 128

    @property
    def W0(self):
        # group boundaries of the G AllGather chunks (13,12,12,12)
        per = -(-self.NG // self.G)
        return [min(self.NG, per * g) for g in range(self.G + 1)]


CFG = Cfg()


# ---------------------------------------------------------------- host prep


def prep(x, edge_index, cfg=CFG):
    """Host-side sharding/schedule. Returns (sched, per-core input maps)."""
    C, G, NG, GRP, NSH, SLOTS = cfg.C, cfg.G, cfg.NG, cfg.GRP, cfg.NSH, cfg.SLOTS
    CALL = cfg.CALL
    W0 = cfg.W0
    src = np.asarray(edge_index[0]).astype(np.int64)
    dst = np.asarray(edge_index[1]).astype(np.int64)

    deg = np.bincount(dst, minlength=cfg.N).astype(np.int64)
    invdeg = (1.0 / np.maximum(deg, 1.0)).astype(np.float32)

    # --- degree-sorted slot assignment per core
    slot_of = np.full(cfg.N, -1, np.int64)     # node -> local slot
    sl_nodes = []                               # per core: slot -> local node
    Rc = np.zeros((C, NG), np.int64)            # per-core max degree per group
    for c in range(C):
        lo = c * NSH
        dl = deg[lo:lo + NSH]
        order = np.argsort(-dl, kind="stable")
        slot_of[lo + order] = np.arange(NSH)
        sn = np.full(SLOTS, -1, np.int64)
        sn[:NSH] = order
        sl_nodes.append(sn)
        dls = np.zeros(SLOTS, np.int64)
        dls[:NSH] = dl[order]
        Rc[c] = dls.reshape(NG, GRP).max(axis=1)
    R = Rc.max(axis=0)                          # shared rounds per group
    NCH1 = int((2 * R).sum())                   # L1 stream chunks
    rowbase = np.concatenate([[0], np.cumsum(R * GRP)]).astype(np.int64)

    # --- mfull chunk-major layout
    crows = np.array([(W0[g + 1] - W0[g]) * GRP for g in range(G)], np.int64)
    cbase = np.concatenate([[0], np.cumsum(crows * C)]).astype(np.int64)
    for g in range(G):
        assert crows[g] * C <= 32767
    gchunk_of_grp = np.zeros(NG, np.int64)
    for g in range(G):
        gchunk_of_grp[W0[g]:W0[g + 1]] = g
    ncore = np.arange(cfg.N) // NSH
    nslot = slot_of
    ngrp = nslot // GRP
    nchunk = gchunk_of_grp[ngrp]
    w0_arr = np.asarray(W0)
    mrow = cbase[nchunk] + ncore * crows[nchunk] + (nslot - w0_arr[nchunk] * GRP)
    mrel = mrow - cbase[nchunk]                 # row within its chunk block

    # --- per-core L2 cell counts (shared K2 = max over cores)
    ecore = dst // NSH
    eg2_all = slot_of[dst] // GRP               # destination group
    eb_all = nchunk[src]                        # source block (= AllGather chunk)
    cnt2c = np.zeros((C, NG, G), np.int64)
    for c in range(C):
        m = ecore == c
        np.add.at(cnt2c[c], (eg2_all[m], eb_all[m]), 1)
    K2 = np.ceil(cnt2c / 128).astype(np.int64).max(axis=0)   # [NG, G]
    NCH2 = int(K2.sum())
    NCH2P = -(-NCH2 // cfg.B2) * cfg.B2
    Sb = (K2.sum(axis=0) * 128).astype(np.int64)             # [G] idx rows
    ncalls = np.ceil(Sb / CALL).astype(np.int64)
    lastvalid = Sb - (ncalls - 1) * CALL
    GOFF = np.concatenate([[0], np.cumsum(ncalls * (CALL // 16))]).astype(int)

    # first block with a nonempty (shared) cell, per group
    first_blk = np.full(NG, -1, np.int64)
    for g2 in range(NG):
        nz = np.nonzero(K2[g2])[0]
        if len(nz):
            first_blk[g2] = nz[0]

    # cell base offsets within each block's idx stream (shared)
    cellbase = np.zeros((NG, G), np.int64)
    for b in range(G):
        off = 0
        for g2 in range(NG):
            cellbase[g2, b] = off
            off += int(K2[g2, b]) * 128
    # drel column index of chunk k of cell (g2, b): block-major order
    dcol0 = np.zeros((G, NG), np.int64)
    off = 0
    for b in range(G):
        for g2 in range(NG):
            dcol0[b, g2] = off
            off += int(K2[g2, b])
    assert off == NCH2

    xq = np.asarray(x, np.float32).astype(bfnp)

    in_maps = []
    for c in range(C):
        lo = c * NSH
        m = ecore == c
        es, ed = src[m], dst[m]
        eslot = slot_of[ed]

        # ---- L1 identity stream
        o1 = np.argsort(eslot, kind="stable")
        es1, eslot1 = es[o1], eslot[o1]
        cnt = np.bincount(eslot1, minlength=SLOTS)
        off1 = np.concatenate([[0], np.cumsum(cnt)])
        seq = np.arange(len(es1)) - off1[eslot1]
        g1 = eslot1 // GRP
        pos1 = eslot1 % GRP
        srow = rowbase[g1] + seq * GRP + pos1
        eidx = np.full(NCH1 * 128, -1, np.int64)
        eidx[srow] = es1
        vals = np.zeros((NCH1 * 128, cfg.D), bfnp)
        real = eidx >= 0
        vals[real] = xq[eidx[real]]
        xe = np.ascontiguousarray(
            vals.reshape(NCH1, 128, cfg.D).transpose(1, 0, 2).reshape(128, -1))

        # ---- L2 gather streams + drel
        eg2 = eslot // GRP
        eb = eb_all[m]
        erel = mrel[es]
        epos = eslot % GRP
        o2 = np.lexsort((erel, eb, eg2))
        eg2s, ebs, erels, eposs = eg2[o2], eb[o2], erel[o2], epos[o2]
        drel = np.full((NCH2P, 128), 300.0, np.float32)
        gstreams = [np.zeros(int(ncalls[b]) * CALL, np.int16) for b in range(G)]
        for b in range(G):
            if Sb[b] < ncalls[b] * CALL:
                gstreams[b][Sb[b]:] = -1
        eoff = 0
        for b in range(G):
            sel = ebs == b
            nb = int(sel.sum())
            g2b = eg2s[sel]
            relb = erels[sel]
            posb_ = eposs[sel]
            # place each cell's edges at its shared base offset
            cnt_b = np.bincount(g2b, minlength=NG)
            offs = np.concatenate([[0], np.cumsum(cnt_b)])
            seq2 = np.arange(nb) - offs[g2b]
            p = cellbase[g2b, b] + seq2
            gstreams[b][p] = relb.astype(np.int16)
            # drel: chunk col = dcol0[b, g2] + seq2//128, row = seq2%128
            dc = dcol0[b, g2b] + seq2 // 128
            drel[dc, seq2 % 128] = posb_
            eoff += nb
        assert eoff == len(es)
        gparts = [gstreams[b].reshape(-1, 16).T for b in range(G)]
        gidx16 = np.concatenate(gparts, axis=1)
        gidx = np.tile(gidx16, (8, 1)).astype(np.int16)

        # ---- slot-ordered tables
        sn = sl_nodes[c]
        realsl = sn >= 0
        invsl = np.ones(SLOTS, np.float32)
        invsl[realsl] = invdeg[lo + sn[realsl]]
        bcast = np.ascontiguousarray(
            np.broadcast_to(invsl.astype(bfnp), (128, SLOTS)))
        xT = np.zeros((cfg.D, SLOTS), bfnp)
        xT[:, realsl] = xq[lo + sn[realsl]].T

        in_maps.append(dict(
            xe=xe,
            gidx=gidx,
            drel=np.ascontiguousarray(drel.T.astype(np.float32)),
            drelb=np.ascontiguousarray(drel.T.astype(bfnp)),
            bcast=bcast,
            xT=np.ascontiguousarray(xT),
            slot_nodes=sn,                     # host-only
        ))

    sched = dict(R=R, K2=K2, NCH1=NCH1, NCH2=NCH2, NCH2P=NCH2P,
                 ncalls=ncalls, lastvalid=lastvalid, GOFF=GOFF,
                 cbase=cbase, crows=crows, first_blk=first_blk,
                 TCH=NCH1 + NCH2)
    return sched, in_maps


def make_shared(inputs, cfg=CFG):
    """Weight products + bias rows shipped to every core."""
    f8d = np.float64
    Wl1 = np.asarray(inputs["Wl1"], f8d)
    Wr1 = np.asarray(inputs["Wr1"], f8d)
    Wl2 = np.asarray(inputs["Wl2"], f8d)
    Wr2 = np.asarray(inputs["Wr2"], f8d)
    b1 = np.asarray(inputs["b1"], f8d)
    b2 = np.asarray(inputs["b2"], f8d)
    WaT = (Wl2 @ Wl1).T
    WbT = (Wl2 @ Wr1 + Wr2 @ Wl1).T
    WcT = (Wr2 @ Wr1).T
    c0 = b2 + Wr2 @ b1
    c1 = Wl2 @ b1
    return dict(
        wat=np.ascontiguousarray(WaT.astype(bfnp)),
        wbt=np.ascontiguousarray(WbT.astype(bfnp)),
        wct=np.ascontiguousarray(WcT.astype(bfnp)),
        c0r=np.ascontiguousarray(c0[None, :].astype(bfnp)),
        c1r=np.ascontiguousarray(c1[None, :].astype(bfnp)),
    ), bool(np.abs(c0).max() > 0 or np.abs(c1).max() > 0)


# ---------------------------------------------------------------- program


def build(cfg, sched, has_bias=False):
    R, K2 = sched["R"], sched["K2"]
    NCH1, NCH2P = sched["NCH1"], sched["NCH2P"]
    ncalls, lastvalid, GOFF = sched["ncalls"], sched["lastvalid"], sched["GOFF"]
    cbase, first_blk = sched["cbase"], sched["first_blk"]
    C, D, G, NG, GRP = cfg.C, cfg.D, cfg.G, cfg.NG, cfg.GRP
    CALL, CALLCH, SLOTS = cfg.CALL, cfg.CALLCH, cfg.SLOTS
    XEG, B2, SCAL_EVERY = cfg.XEG, cfg.B2, cfg.SCAL_EVERY
    SCW, OCW = cfg.SCW, cfg.OCW
    W0 = cfg.W0

    nc = bacc.Bacc(None, num_devices=C, num_swdge_queues=4,
                   dynamic_dma_scratch_size=49152)
    xe_d = nc.dram_tensor("xe", [128, NCH1 * 128], BF, kind="ExternalInput")
    xT_d = nc.dram_tensor("xT", [D, SLOTS], BF, kind="ExternalInput")
    gidx_d = nc.dram_tensor("gidx", [128, int(GOFF[-1])], I16, kind="ExternalInput")
    drel_d = nc.dram_tensor("drel", [128, NCH2P], F32, kind="ExternalInput")
    drelb_d = nc.dram_tensor("drelb", [128, NCH2P], BF, kind="ExternalInput")
    bc_d = nc.dram_tensor("bcast", [128, SLOTS], BF, kind="ExternalInput")
    w_d = {nm: nc.dram_tensor(nm, [D, D], BF, kind="ExternalInput")
           for nm in ("wat", "wbt", "wct")}
    c0_d = nc.dram_tensor("c0r", [1, D], BF, kind="ExternalInput")
    c1_d = nc.dram_tensor("c1r", [1, D], BF, kind="ExternalInput")
    out_d = nc.dram_tensor("out", [SLOTS, D], BF, kind="ExternalOutput")

    ident_d = nc.inline_tensor(np.eye(128, dtype=bfnp), "identc")
    iota_d = nc.inline_tensor(
        np.broadcast_to(np.arange(GRP, dtype=bfnp), (128, GRP)).copy(), "iotac")

    msl_d = nc.dram_tensor("mslots", [SLOTS, D], BF)
    mfull_d = nc.dram_tensor("mfull", [C * SLOTS, D], BF, addr_space="Shared")

    with tile.TileContext(nc) as tc, ExitStack() as ctx:
        const = ctx.enter_context(tc.tile_pool(name="const", bufs=1))
        meta = ctx.enter_context(tc.tile_pool(name="meta", bufs=1))
        gxp = ctx.enter_context(tc.tile_pool(name="gx", bufs=4))
        xep = ctx.enter_context(tc.tile_pool(name="xe", bufs=2))
        gpool = ctx.enter_context(tc.tile_pool(name="gather", bufs=8))
        ohp = ctx.enter_context(tc.tile_pool(name="oh", bufs=3))
        ddp = ctx.enter_context(tc.tile_pool(name="dd", bufs=2))
        mwp = ctx.enter_context(tc.tile_pool(name="mw", bufs=4))
        msp = ctx.enter_context(tc.tile_pool(name="ms", bufs=1))
        map_ = ctx.enter_context(tc.tile_pool(name="macc", bufs=1))
        bcp = ctx.enter_context(tc.tile_pool(name="bc", bufs=1))
        xtp = ctx.enter_context(tc.tile_pool(name="xt", bufs=2))
        stgp = ctx.enter_context(tc.tile_pool(name="stg", bufs=2))
        ostgp = ctx.enter_context(tc.tile_pool(name="ostg", bufs=2))
        psA = ctx.enter_context(tc.tile_pool(name="psA", bufs=4, space="PSUM"))
        psP = ctx.enter_context(tc.tile_pool(name="psP", bufs=2, space="PSUM"))
        psB = ctx.enter_context(tc.tile_pool(name="psB", bufs=1, space="PSUM"))
        psT = ctx.enter_context(tc.tile_pool(name="psT", bufs=1, space="PSUM"))

        def load(pool, dram, shape, dtype):
            t = pool.tile(shape, dtype, tag=dram.name)
            nc.sync.dma_start(t[:], dram[:])
            return t

        ident_s = load(const, ident_d, [128, 128], BF)
        iota_s = load(const, iota_d, [128, GRP], BF)
        w_s = {nm: load(const, w_d[nm], [D, D], BF) for nm in w_d}
        if has_bias:
            ones_d = nc.inline_tensor(np.ones((1, 128), bfnp), "onesc")
            ones_s = load(const, ones_d, [1, 128], BF)
            c0_s = load(const, c0_d, [1, D], BF)
            c1_s = load(const, c1_d, [1, D], BF)
            mask_d = nc.dram_tensor("maskr", [1, SLOTS], BF, kind="ExternalInput")
            mask_s = load(const, mask_d, [1, SLOTS], BF)
        drel_s = load(meta, drel_d, [128, NCH2P], F32)
        drelb_s = load(meta, drelb_d, [128, NCH2P], BF)
        bc_s = load(bcp, bc_d, [128, SLOTS], BF)

        gx_tiles = {}
        for _b in range(G):
            _t = gxp.tile([128, int(GOFF[_b + 1] - GOFF[_b])], I16, tag="gx",
                          name=f"gx{_b}")
            nc.sync.dma_start(_t[:], gidx_d[:, int(GOFF[_b]):int(GOFF[_b + 1])])
            gx_tiles[_b] = _t

        m_s = msp.tile([D, SLOTS], BF, tag="ms")
        macc = map_.tile([128, SLOTS], BF, tag="macc")

        # ---------------- L1 + chunked AllGather
        st = dict(xch=0, xet=None, stg=None, w=0)

        def l1_group(g):
            gsl = slice(g * GRP, (g + 1) * GRP)
            Rg = int(R[g])
            if Rg == 0:
                nc.vector.memset(m_s[:, gsl], 0.0)
            else:
                psums = [psA.tile([128, 128], F32, tag="agg",
                                  name=f"agg{g}_{h}") for h in (0, 1)]
                for r in range(Rg):
                    for half in (0, 1):
                        ci = st["xch"]
                        xgi, xgo = divmod(ci, XEG)
                        if xgo == 0:
                            nx = min(XEG, NCH1 - xgi * XEG)
                            xet = xep.tile([128, XEG, 128], BF, tag="xe",
                                           name=f"xe{xgi}")
                            nc.sync.dma_start(
                                xet[:, :nx, :],
                                xe_d[:, xgi * XEG * 128:(xgi * XEG + nx) * 128])
                            st["xet"] = xet
                        nc.tensor.matmul(
                            out=psums[half][:], lhsT=st["xet"][:, xgo, :],
                            rhs=ident_s[:], start=(r == 0), stop=(r == Rg - 1))
                        st["xch"] += 1
                for half in (0, 1):
                    wsl = slice(g * GRP + half * 128, g * GRP + (half + 1) * 128)
                    nc.vector.tensor_tensor(
                        out=m_s[:, wsl], in0=psums[half][:], in1=bc_s[:, wsl],
                        op=mybir.AluOpType.mult)

        def l1_store(g, w_hi):
            # transpose m for the two 128-windows of group g and stage out
            for half in (0, 1):
                w = st["w"]
                wsl = slice(w * 128, (w + 1) * 128)
                pst = psT.tile([128, 128], BF, tag="tr", name=f"tr{w}")
                nc.tensor.transpose(pst[:], m_s[:, wsl], ident_s[:])
                wi = w % SCW
                if wi == 0 or st["stg"] is None:
                    st["stg"] = stgp.tile([128, SCW, 128], BF, tag="stg",
                                          name=f"stg{w}")
                    st["stg_w0"] = w
                si = w - st["stg_w0"]
                nc.scalar.copy(st["stg"][:, si, :], pst[:])
                if wi == SCW - 1 or w == w_hi - 1:
                    used = si + 1
                    w0 = st["stg_w0"]
                    hap = msl_d[:].rearrange("(w p) f -> p w f", p=128)
                    nc.sync.dma_start(hap[:, w0:w0 + used, :],
                                      st["stg"][:, :used, :])
                    st["stg"] = None
                st["w"] += 1

        for gc in range(G):
            w_hi = W0[gc + 1] * 2
            for g in range(W0[gc], W0[gc + 1]):
                l1_group(g)
                l1_store(g, w_hi)
            r0, r1 = W0[gc] * GRP, W0[gc + 1] * GRP
            nc.gpsimd.collective_compute(
                "AllGather", mybir.AluOpType.bypass,
                replica_groups=[list(range(C))],
                ins=[msl_d[r0:r1, :]],
                outs=[mfull_d[int(cbase[gc]):int(cbase[gc + 1]), :]],
            )

        # ---------------- L2
        st2 = dict(posb=[0] * G, gt=None, sct=0, stile=None, ostg=None,
                   xtt=None, ostg_w0=0)

        def one_hot(sc_idx):
            si, sc = divmod(sc_idx, B2)
            if sc == 0:
                if SCAL_EVERY and si % SCAL_EVERY == SCAL_EVERY - 1:
                    # scalar-engine batch: per-chunk Abs/Relu pair
                    stile = ohp.tile([128, B2, GRP], BF, tag="ohs",
                                     name=f"ohs{si}")
                    for j in range(B2):
                        dcol = slice(si * B2 + j, si * B2 + j + 1)
                        dd = ddp.tile([128, GRP], BF, tag="dd",
                                      name=f"dd{si}_{j}")
                        nc.scalar.activation(dd[:], iota_s[:],
                                             mybir.ActivationFunctionType.Abs,
                                             bias=drel_s[:, dcol], scale=-1.0)
                        nc.scalar.activation(stile[:, j, :], dd[:],
                                             mybir.ActivationFunctionType.Relu,
                                             bias=1.0, scale=-1.0)
                else:
                    stile = ohp.tile([128, B2, GRP], BF, tag="ohv",
                                     name=f"ohv{si}")
                    in0 = iota_s[:].rearrange("p (b w) -> p b w", b=1) \
                        .broadcast_to([128, B2, GRP])
                    in1 = drelb_s[:, si * B2:(si + 1) * B2] \
                        .rearrange("p (b o) -> p b o", o=1) \
                        .broadcast_to([128, B2, GRP])
                    nc.vector.tensor_tensor(out=stile[:], in0=in0, in1=in1,
                                            op=mybir.AluOpType.is_equal)
                st2["stile"] = stile
            return st2["stile"][:, sc, :]

        for b in range(G):
            gxt = gx_tiles.pop(b)
            for g2 in range(NG):
                gsl = slice(g2 * GRP, (g2 + 1) * GRP)
                kk = int(K2[g2, b])
                if kk:
                    psum = psP.tile([128, GRP], F32, tag="mm",
                                    name=f"mm{b}_{g2}")
                    for k in range(kk):
                        pos = st2["posb"][b]
                        call_i, col = divmod(pos, CALLCH)
                        if col == 0:
                            gt = gpool.tile([128, CALLCH, 128], BF, tag="g",
                                            name=f"g{b}_{call_i}")
                            nvalid = (CALL if call_i < int(ncalls[b]) - 1
                                      else int(lastvalid[b]))
                            nc.gpsimd.dma_gather(
                                out_ap=gt[:],
                                in_ap=mfull_d[int(cbase[b]):int(cbase[b + 1]), :],
                                idxs_ap=gxt[:, call_i * (CALL // 16):
                                            (call_i + 1) * (CALL // 16)],
                                num_idxs=CALL,
                                num_idxs_reg=nvalid,
                                elem_size=D,
                            )
                            st2["gt"] = gt
                        rhs = one_hot(st2["sct"])
                        nc.tensor.matmul(
                            out=psum[:], lhsT=st2["gt"][:, col, :], rhs=rhs,
                            start=(k == 0), stop=(k == kk - 1))
                        st2["sct"] += 1
                        st2["posb"][b] += 1
                    if b == int(first_blk[g2]):
                        nc.scalar.copy(macc[:, gsl], psum[:])
                    else:
                        nc.vector.tensor_tensor(
                            out=macc[:, gsl], in0=psum[:], in1=macc[:, gsl],
                            op=mybir.AluOpType.add)
                if b == G - 1:
                    # finalize group: mm = macc * invdeg, then transform
                    if int(first_blk[g2]) < 0:
                        nc.vector.memset(macc[:, gsl], 0.0)
                    mm = mwp.tile([128, GRP], BF, tag="mw", name=f"mw{g2}")
                    nc.vector.tensor_tensor(out=mm[:], in0=macc[:, gsl],
                                            in1=bc_s[:, gsl],
                                            op=mybir.AluOpType.mult)
                    for half in (0, 1):
                        w2 = g2 * 2 + half
                        wsl = slice(w2 * 128, (w2 + 1) * 128)
                        xgi, xgo = divmod(w2, 8)
                        if xgo == 0 or st2["xtt"] is None:
                            nxw = min(8, 2 * NG - xgi * 8)
                            xtt = xtp.tile([128, 8, 128], BF, tag="xT",
                                           name=f"xT{xgi}")
                            nc.sync.dma_start(
                                xtt[:, :nxw, :],
                                xT_d[:, xgi * 1024:xgi * 1024 + nxw * 128])
                            st2["xtt"] = xtt
                        pso = psB.tile([128, 128], F32, tag="o", name=f"o{w2}")
                        nc.tensor.matmul(out=pso[:], lhsT=mm[:, half * 128:(half + 1) * 128],
                                         rhs=w_s["wat"][:], start=True, stop=False)
                        nc.tensor.matmul(out=pso[:], lhsT=m_s[:, wsl],
                                         rhs=w_s["wbt"][:], start=False, stop=False)
                        nc.tensor.matmul(out=pso[:], lhsT=st2["xtt"][:, xgo, :],
                                         rhs=w_s["wct"][:], start=False,
                                         stop=not has_bias)
                        if has_bias:
                            nc.tensor.matmul(out=pso[:], lhsT=ones_s[0:1, :],
                                             rhs=c0_s[0:1, :], start=False,
                                             stop=False)
                            nc.tensor.matmul(out=pso[:], lhsT=mask_s[0:1, wsl],
                                             rhs=c1_s[0:1, :], start=False,
                                             stop=True)
                        wi = w2 % OCW
                        if wi == 0 or st2["ostg"] is None:
                            st2["ostg"] = ostgp.tile([128, OCW, 128], BF,
                                                     tag="ostg", name=f"os{w2}")
                            st2["ostg_w0"] = w2
                        oi = w2 - st2["ostg_w0"]
                        nc.scalar.copy(st2["ostg"][:, oi, :], pso[:])
                        if wi == OCW - 1 or w2 == 2 * NG - 1:
                            used = oi + 1
                            w0_ = st2["ostg_w0"]
                            oap = out_d[:].rearrange("(w p) f -> p w f", p=128)
                            nc.sync.dma_start(oap[:, w0_:w0_ + used, :],
                                              st2["ostg"][:, :used, :])
                            st2["ostg"] = None

    # spread SWDGE gather descriptor generation across the 4 SWDGE queues
    from concourse.tile_sem_assignment import PROC_NAME_TO_IDX
    dmasw0 = PROC_NAME_TO_IDX["DMASW0"]
    for inst in nc.inst_map.values():
        if isinstance(inst, (mybir.InstDMAGatherAnt, mybir.InstDMAScatterAddAnt)):
            proc = getattr(inst, "bass_scheduled_proc", None)
            if proc is not None and dmasw0 <= proc < dmasw0 + 8:
                inst.queue_num = (proc - dmasw0) % 4

    nc.compile()
    return nc


# ---------------------------------------------------------------- kernel


def kernel(**inputs):
    cfg = CFG
    x = np.asarray(inputs["x"], np.float32)
    ei = np.asarray(inputs["edge_index"])
    sched, in_maps = prep(x, ei, cfg)
    shared, has_bias = make_shared(inputs, cfg)
    nc = build(cfg, sched, has_bias=has_bias)

    slot_nodes = [m.pop("slot_nodes") for m in in_maps]
    if has_bias:
        deg = np.bincount(np.asarray(ei[1]).astype(np.int64), minlength=cfg.N)
        for c in range(cfg.C):
            sn = slot_nodes[c]
            mr = np.zeros((1, cfg.SLOTS), bfnp)
            real = sn >= 0
            mr[0, real] = (deg[c * cfg.NSH + sn[real]] > 0).astype(bfnp)
            in_maps[c]["maskr"] = mr
    run_maps = [dict(shared, **m) for m in in_maps]

    res = None
    last_err = None
    for attempt in range(3):
        try:
            res = run_bass_kernel_spmd(nc, run_maps, core_ids=list(range(cfg.C)))
            break
        except Exception as e:  # transient device wedge: retry
            last_err = e
            import time
            time.sleep(10)
    if res is None:
        raise last_err
    out = np.empty((cfg.N, cfg.D), np.float32)
    for c in range(cfg.C):
        oc = res.results[c]["out"]
        sn = slot_nodes[c]
        real = sn >= 0
        out[c * cfg.NSH + sn[real]] = oc[real]
    return out


if __name__ == "__main__":
    d = np.load("/tmp/inputs.npz")
    ins = {k: d[k] for k in ("x", "edge_index", "Wl1", "Wr1", "b1", "Wl2", "Wr2", "b2")}
    got = kernel(**ins)
    exp = d["expected"]
    err = np.abs(got - exp).max() / np.abs(exp).max()
    print("Relative error:", err)
